# revision 34
# baseline (speedup 1.0000x reference)
"""GCN+GIN graph encoder on 8 Trainium2 NeuronCores (Bass/Tile).

Math (reference):
  GCNConv:  h = relu(segsum_dst(norm_e * (x@W0)[src]) + b0),
            norm_e = dinv[src]*dinv[dst] over edges+self-loops,
            dinv = rsqrt(deg incl self-loop)
  GIN x2:   h = relu((h + segsum_dst(h[src])) @ Wg + bg)
  pool:     m = segment_mean(h, batch) -> relu(m@Wh1+bh1)@Wh2+bh2

Distribution: nodes (and their in-edges) sharded contiguously over 8 cores.
Per layer each core aggregates messages for its own dst nodes by gathering
rows of a replicated bf16 node-feature table (dma_gather on 4 SWDGE queues),
reducing edge tiles with one-hot selection matrices on the TensorEngine,
applying the layer linear transform W-stationary in feat-major, then
transposing back to node-major.  Tables are re-replicated between layers
with an AllGather; pooled partials are combined with an AllReduce and the
small MLP head is computed redundantly on every core.

Harness-cost driven design:
 * per-exec NEFF staging scales with STATIC instruction count -> each
   layer's per-block work runs in a single For_i hardware loop (every
   block padded to the same per-stream tile count), keeping the static
   program ~800 instructions instead of ~12k fully unrolled.
 * per-exec input shipping costs ~25 ms/MB and ~10 ms/array -> all
   per-core inputs are packed into ONE uint8 blob (bitcast-viewed on
   device): x int8 with per-feature scales, edge streams idx:int16 /
   doff:int8, weights bf16.  Each core returns only its own 32 graphs
   (partition-id-sliced output), reassembled on host.

Aggregation identity per dst block b (128 dst nodes):
  aggT[f, d] = sum_e msg[e, f] * sel[e, d],  sel[e, d] = (doff[e] == d)
computed as matmul(lhsT=msg_tile[128e, 128f], rhs=sel[128e, 128d]) accumulated
in PSUM over the block's edge tiles (one-hots built 4 tiles per DVE op).
GCN's sym-norm is applied without per-edge values: dinv[src] is folded into
the x rows on host and the int8 dequant scale s[f] plus the dinv[dst] factor
are fused into one scalar_tensor_tensor on the aggregation PSUM.  GIN needs
no normalisation and a self-loop edge supplies the "+h" term.  Pad edge
slots carry doff=-1 -> zero contribution.
"""
import sys

sys.path.insert(0, '/opt/trn_rl_repo')

import numpy as np
import ml_dtypes

import concourse.bass as bass
import concourse.bacc as bacc
import concourse.mybir as mybir
import concourse.tile as tile
from concourse.bass import ds, ts
from concourse.bass_utils import run_bass_kernel_spmd
from concourse.masks import make_identity

F32 = mybir.dt.float32
BF16 = mybir.dt.bfloat16
I16 = mybir.dt.int16
I8 = mybir.dt.int8
U8 = mybir.dt.uint8
BF = ml_dtypes.bfloat16
P = 128
NCORES = 8
GMAX = 1024                 # max rows per dma_gather (single_packet limit)
NQ = 4                      # SWDGE queues
ALIGN = 512
# bf16 weight sections shipped only on core 0 (zeros elsewhere compress on
# the wire) and broadcast on device via a bit-exact bf16 AllReduce-add
BCAST_W = ("w0", "wg1", "wg2", "wh1", "wh2pack")


class Cfg:
    def __init__(self, N, E, G, F, NHID, NOUT, NPN):
        self.N = N            # real nodes
        self.E = E            # edges (no self loops)
        self.G = G            # graphs
        self.F = F            # feature/hidden width (128)
        self.NHID = NHID
        self.NOUT = NOUT
        self.NPN = NPN        # real nodes per core
        assert NPN * NCORES >= N > NPN * (NCORES - 1)
        self.NPC = ((NPN + P - 1) // P) * P   # padded nodes per core
        self.NBLK = self.NPC // P
        self.NPAD = self.NPC * NCORES
        self.NHALF = self.NPAD // 2
        assert self.NHALF < 32768
        assert G == 2 * P


FULL = Cfg(N=50000, E=800000, G=256, F=128, NHID=256, NOUT=128, NPN=6250)


# ---------------------------------------------------------------- host prep
def preprocess(cfg, x, edge_index, batch, W0, b0, Wg1, bg1, Wg2, bg2,
               Wh1, bh1, Wh2, bh2):
    N, G, F = cfg.N, cfg.G, cfg.F
    NPN, NPC, NBLK, NHALF = cfg.NPN, cfg.NPC, cfg.NBLK, cfg.NHALF

    src = np.asarray(edge_index[0], dtype=np.int64)
    dst = np.asarray(edge_index[1], dtype=np.int64)
    batch = np.asarray(batch, dtype=np.int64)
    loop = np.arange(N, dtype=np.int64)
    s_all = np.concatenate([src, loop])
    d_all = np.concatenate([dst, loop])

    deg = np.bincount(d_all, minlength=N).astype(np.float64)
    dinv = (1.0 / np.sqrt(np.maximum(deg, 1.0))).astype(np.float32)

    def tabidx(n):
        c = n // NPN
        return c * NPC + (n - c * NPN)

    sidx = tabidx(s_all).astype(np.int64)
    c_e = d_all // NPN
    loc = d_all - c_e * NPN
    b_e = loc // P
    off_e = loc % P
    gblk = c_e * NBLK + b_e                      # global dst block id
    val_e = dinv[d_all].astype(np.float32)      # GCN dst scaling

    NGB = NCORES * NBLK
    streams = {}
    for name, mask in (("lo", sidx < NHALF), ("hi", sidx >= NHALF)):
        sg = gblk[mask]
        si = sidx[mask] - (0 if name == "lo" else NHALF)
        sof = off_e[mask]
        order = np.argsort(sg, kind="stable")
        sg, si, sof = sg[order], si[order], sof[order]
        cnt = np.bincount(sg, minlength=NGB)
        # uniform per-block tile count (same For_i body for every block/core)
        NT = int(np.ceil(cnt.max() / P))
        rows_blk = NT * P
        rows_core = NBLK * rows_blk
        starts = np.zeros(NGB, dtype=np.int64)
        starts[1:] = np.cumsum(cnt)[:-1]
        rank = np.arange(len(sg)) - np.repeat(starts, cnt)
        c_of = sg // NBLK
        b_of = sg % NBLK
        pos = c_of * rows_core + b_of * rows_blk + rank
        tot = NCORES * rows_core
        idx_arr = np.zeros(tot, dtype=np.int32)
        doff_arr = np.full(tot, -1, dtype=np.int8)
        idx_arr[pos] = si
        doff_arr[pos] = sof
        idx_arr = idx_arr.reshape(NCORES, NBLK, rows_blk)
        # gather chunk sizes within a block: full 1024s then the remainder
        chunks = [GMAX] * (rows_blk // GMAX)
        if rows_blk % GMAX:
            chunks.append(rows_blk % GMAX)
        cols_blk = rows_blk // 16
        wrapped = np.zeros((NCORES, 16, NBLK * cols_blk), dtype=np.int16)
        for b in range(NBLK):
            a = 0
            cc = b * cols_blk
            for sz in chunks:
                wrapped[:, :, cc:cc + sz // 16] = (
                    idx_arr[:, b, a:a + sz].reshape(NCORES, sz // 16, 16)
                    .swapaxes(1, 2))
                a += sz
                cc += sz // 16
        T = NBLK * NT
        doff2 = doff_arr.reshape(NCORES, T, P).transpose(0, 2, 1).copy()
        streams[name] = dict(NT=NT, T=T, chunks=chunks,
                             idx=wrapped, doff=doff2)

    # per-core node features: dinv[src]-prescaled, int8 with per-feature
    # scales (dequant s[f] and the dinv[dst] factor are folded into the
    # aggregation epilogue on device).  int4 was tried and fails the 2e-2
    # max-abs gate (2.96e-2): tail outliers dominate, don't retry.
    x = np.asarray(x, dtype=np.float32)
    xt = x * dinv[:, None]
    s_feat = (np.abs(xt).max(axis=0) / 127.0).astype(np.float32)  # [F]
    xq_full = np.clip(np.round(xt / s_feat[None, :]), -127, 127).astype(np.int8)
    xs = np.zeros((NCORES, NPC, F), dtype=np.int8)
    dinv_rows = np.zeros((NCORES, 1, NPC), dtype=np.float32)
    for c in range(NCORES):
        lo_n = c * NPN
        hi_n = min(N, (c + 1) * NPN)
        n = hi_n - lo_n
        xs[c, :n] = xq_full[lo_n:hi_n]
        dinv_rows[c, 0, :n] = dinv[lo_n:hi_n]

    # pooling metadata: batch id per node, block-column-major, pad=-1
    cnt_g = np.bincount(batch, minlength=G).astype(np.float32)
    invc = (1.0 / np.maximum(cnt_g, 1.0)).astype(np.float32)
    bat = np.full((NCORES, P, NBLK), -1.0, dtype=BF)
    for c in range(NCORES):
        lo_n = c * NPN
        hi_n = min(N, (c + 1) * NPN)
        n = hi_n - lo_n
        colmaj = np.full(NPC, -1.0, dtype=np.float32)
        colmaj[:n] = batch[lo_n:hi_n].astype(np.float32)
        bat[c] = colmaj.reshape(NBLK, P).T.astype(BF)

    # weights bf16; wh2 packed [P, 2*NOUT] (chunk h at cols h*NOUT)
    wh2 = np.asarray(Wh2, np.float32)
    wh2pack = np.concatenate([wh2[0:P, :], wh2[P:2 * P, :]], axis=1).astype(BF)
    # f32 bias columns [P, 6]: b0, bg1, bg2, bh1_0, bh1_1, bh2
    bh1 = np.asarray(bh1, np.float32)
    bcols = np.stack([
        np.asarray(b0, np.float32), np.asarray(bg1, np.float32),
        np.asarray(bg2, np.float32), bh1[0:P].reshape(P), bh1[P:2 * P].reshape(P),
        np.asarray(bh2, np.float32)], axis=1).copy()

    common = [
        ("w0", np.asarray(W0, np.float32).astype(BF)),
        ("wg1", np.asarray(Wg1, np.float32).astype(BF)),
        ("wg2", np.asarray(Wg2, np.float32).astype(BF)),
        ("wh1", np.asarray(Wh1, np.float32).astype(BF)),
        ("wh2pack", wh2pack),
        ("bcols", bcols),
        ("invc", invc.reshape(1, G)),
        ("sfeat", s_feat.reshape(1, F)),
    ]

    # ---- pack per-core blobs
    sections = [
        ("xs", None), ("idxlo", None), ("idxhi", None),
        ("dofflo", None), ("doffhi", None), ("dinvrow", None),
        ("bat", None),
    ] + common
    percore = {
        "xs": xs,
        "idxlo": streams["lo"]["idx"], "idxhi": streams["hi"]["idx"],
        "dofflo": streams["lo"]["doff"], "doffhi": streams["hi"]["doff"],
        "dinvrow": dinv_rows,
        "bat": bat,
    }
    offs, off = {}, 0
    for nm, arr in sections:
        a = percore[nm][0] if arr is None else arr
        offs[nm] = off
        off += (a.nbytes + ALIGN - 1) // ALIGN * ALIGN
    BLOB = off
    blobs = np.zeros((NCORES, BLOB), np.uint8)
    for nm, arr in sections:
        for c in range(NCORES):
            if arr is not None and c > 0 and nm in BCAST_W:
                continue    # core 0 ships the bf16 weights; rest get zeros
            a = percore[nm][c] if arr is None else arr
            raw = np.frombuffer(np.ascontiguousarray(a).tobytes(), np.uint8)
            blobs[c, offs[nm]:offs[nm] + raw.size] = raw

    in_maps = [dict(blob=blobs[c:c + 1]) for c in range(NCORES)]
    meta = dict(NTLO=streams["lo"]["NT"], CHLO=streams["lo"]["chunks"],
                NTHI=streams["hi"]["NT"], CHHI=streams["hi"]["chunks"],
                BLOB=BLOB, offs=offs)
    return in_maps, meta


# ---------------------------------------------------------------- program
def build_program(cfg, meta):
    NPC, NBLK, NPAD, NHALF = cfg.NPC, cfg.NBLK, cfg.NPAD, cfg.NHALF
    F, NHID, NOUT, G = cfg.F, cfg.NHID, cfg.NOUT, cfg.G
    NTLO, CHLO = meta["NTLO"], meta["CHLO"]
    NTHI, CHHI = meta["NTHI"], meta["CHHI"]
    TLO, THI = NBLK * NTLO, NBLK * NTHI
    CLO, CHI = TLO * 8, THI * 8          # idx cols (= rows/16) per core
    BLOB, offs = meta["BLOB"], meta["offs"]

    nc = bacc.Bacc(None, target_bir_lowering=False, debug=True,
                   num_devices=NCORES, num_swdge_queues=NQ)

    GPC = G // NCORES        # graphs output per core (partition-id sliced)
    blob_d = nc.declare_dram_parameter("blob", [1, BLOB], U8, isOutput=False)
    out_d = nc.declare_dram_parameter("out", [GPC, NOUT], F32, isOutput=True)

    def view(nm, dt, rows, cols):
        esz = mybir.dt.size(dt)
        bc = blob_d.bitcast(dt)
        s = offs[nm] // esz
        return bc[0:1, s:s + rows * cols].rearrange("o (r c) -> (o r) c", c=cols)

    # weight-broadcast region: [w0 .. wh2pack], bf16, 512B-aligned sections
    WOFF = offs[BCAST_W[0]]
    W2 = (offs["bcols"] - WOFF) // 2     # bf16 elements
    wsh_in = nc.dram_tensor("wsh_in", [1, W2], BF16)
    wsh_out = nc.dram_tensor("wsh_out", [1, W2], BF16, addr_space="Shared")

    def view_w(nm, rows, cols):
        s = (offs[nm] - WOFF) // 2
        return wsh_out[0:1, s:s + rows * cols].rearrange(
            "o (r c) -> (o r) c", c=cols)

    slice0 = nc.dram_tensor("slice0", [NPC, F], BF16)
    slice1 = nc.dram_tensor("slice1", [NPC, F], BF16)
    slice2 = nc.dram_tensor("slice2", [NPC, F], BF16)
    tab1 = nc.dram_tensor("tab1", [NPAD, F], BF16)
    tab2 = nc.dram_tensor("tab2", [NPAD, F], BF16)
    tab3 = nc.dram_tensor("tab3", [NPAD, F], BF16)
    pool_in = nc.dram_tensor("pool_in", [P, G], F32)
    pool_out = nc.dram_tensor("pool_out", [P, G], F32, addr_space="Shared")
    groups = [list(range(NCORES))]

    with tile.TileContext(nc) as tc:
        with (
            tc.tile_pool(name="const", bufs=1) as constp,
            tc.tile_pool(name="meta", bufs=1) as metap,
            tc.tile_pool(name="msg", bufs=2) as msgp,
            tc.tile_pool(name="sel", bufs=4) as selp,
            tc.tile_pool(name="work", bufs=4) as workp,
            tc.tile_pool(name="pagg", bufs=1, space="PSUM") as pagg,
            tc.tile_pool(name="phT", bufs=1, space="PSUM") as phT,
            tc.tile_pool(name="ptr", bufs=1, space="PSUM") as ptr,
            tc.tile_pool(name="ppool", bufs=1, space="PSUM") as ppool,
            tc.tile_pool(name="phead", bufs=1, space="PSUM") as phead,
        ):
            # broadcast core-0's bf16 weights to all cores (bit-exact:
            # bf16 x + 0.0 == x for the normal-range weight values)
            nc.sync.dma_start(
                out=wsh_in[:],
                in_=blob_d.bitcast(BF16)[0:1, WOFF // 2:WOFF // 2 + W2])
            nc.gpsimd.collective_compute(
                "AllReduce", mybir.AluOpType.add, replica_groups=groups,
                ins=[wsh_in[:]], outs=[wsh_out[:]])

            # ---- constants / metadata to SBUF
            ident = constp.tile([P, P], F32)
            make_identity(nc, ident[:])
            iota = constp.tile([P, P], F32, tag="iota")
            nc.gpsimd.iota(iota[:], pattern=[[1, P]], base=0,
                           channel_multiplier=0,
                           allow_small_or_imprecise_dtypes=True)
            iota4 = constp.tile([P, 4 * P], F32, tag="iota4")
            for k in range(4):
                nc.vector.tensor_copy(out=iota4[:, k * P:(k + 1) * P],
                                      in_=iota[:])
            iotaG = constp.tile([P, G], BF16, tag="iotaG")
            nc.gpsimd.iota(iotaG[:], pattern=[[1, G]], base=0,
                           channel_multiplier=0,
                           allow_small_or_imprecise_dtypes=True)

            def load(nm, t_shape, dt=BF16, pool=metap):
                t = pool.tile(list(t_shape), dt, name=f"sb_{nm}", tag=f"sb_{nm}")
                nc.sync.dma_start(out=t[:], in_=view(nm, dt, *t_shape))
                return t

            # idx tables: ship [16, C], replicate to 128 partitions on device
            idxlo = metap.tile([P, CLO], I16, tag="idxlo")
            idxhi = metap.tile([P, CHI], I16, tag="idxhi")
            for k in range(8):
                nc.sync.dma_start(out=idxlo[16 * k:16 * (k + 1), :],
                                  in_=view("idxlo", I16, 16, CLO))
                nc.sync.dma_start(out=idxhi[16 * k:16 * (k + 1), :],
                                  in_=view("idxhi", I16, 16, CHI))
            dofflo8 = load("dofflo", [P, TLO], I8)
            doffhi8 = load("doffhi", [P, THI], I8)
            dofflo = metap.tile([P, TLO], F32, tag="dofflo_f")
            doffhi = metap.tile([P, THI], F32, tag="doffhi_f")
            nc.vector.tensor_copy(out=dofflo[:], in_=dofflo8[:])
            nc.vector.tensor_copy(out=doffhi[:], in_=doffhi8[:])
            def loadw(nm, t_shape):
                t = constp.tile(list(t_shape), BF16, name=f"sb_{nm}",
                                tag=f"sb_{nm}")
                nc.sync.dma_start(out=t[:], in_=view_w(nm, *t_shape))
                return t

            w0 = loadw("w0", [F, F])
            wg1 = loadw("wg1", [F, F])
            wg2 = loadw("wg2", [F, F])
            wh1 = loadw("wh1", [F, NHID])
            wh2 = loadw("wh2pack", [P, 2 * NOUT])
            bcols = load("bcols", [P, 6], F32, pool=constp)
            batb = load("bat", [P, NBLK], pool=constp)
            bat = constp.tile([P, NBLK], F32, tag="bat_f")
            nc.vector.tensor_copy(out=bat[:], in_=batb[:])
            # invc broadcast [P, G] via rank-1 outer product ones x invc
            ones1 = constp.tile([1, P], F32, tag="ones1")
            nc.any.memset(ones1[:], 1.0)
            invc_row = load("invc", [1, G], F32, pool=constp)
            invb_ps = phead.tile([P, G], F32, space="PSUM", tag="ghead0")
            nc.tensor.matmul(out=invb_ps[:], lhsT=ones1[:], rhs=invc_row[:],
                             start=True, stop=True)
            invc_rep = constp.tile([P, G], F32, tag="invc_rep")
            nc.vector.tensor_copy(out=invc_rep[:], in_=invb_ps[:])
            # s_feat column [F, 1]: outer product s_row x [1]
            sfeat_row = load("sfeat", [1, F], F32, pool=constp)
            one11 = constp.tile([1, 1], F32, tag="one11")
            nc.any.memset(one11[:], 1.0)
            sc_ps = ptr.tile([P, P], F32, space="PSUM", tag="tr")
            nc.tensor.matmul(out=sc_ps[:, 0:1], lhsT=sfeat_row[:], rhs=one11[:],
                             start=True, stop=True)
            s_col = constp.tile([P, 1], F32, tag="s_col")
            nc.vector.tensor_copy(out=s_col[:], in_=sc_ps[:, 0:1])
            dinv_row = load("dinvrow", [1, NPC], F32, pool=constp)
            dinvrep = constp.tile([P, NPC], F32, tag="dinvrep")

            # stage xs: int8 -> bf16 into slice0, build dinvrep alongside
            with tc.For_i(0, NBLK, name="xstage") as i:
                xq = workp.tile([P, F], I8, tag="xq8")
                nc.sync.dma_start(
                    out=xq[:],
                    in_=view("xs", I8, NPC, F)[ts(i, P), :])
                xb = workp.tile([P, F], BF16, tag="xq_bf")
                nc.vector.tensor_copy(out=xb[:], in_=xq[:])
                nc.sync.dma_start(out=slice0[ts(i, P), :], in_=xb[:])
                dv_ps = ptr.tile([P, P], F32, space="PSUM", tag="tr")
                nc.tensor.matmul(out=dv_ps[:], lhsT=ones1[:],
                                 rhs=dinv_row[0:1, ts(i, P)],
                                 start=True, stop=True)
                nc.vector.tensor_copy(out=dinvrep[:, ts(i, P)], in_=dv_ps[:])
            nc.gpsimd.collective_compute(
                "AllGather", mybir.AluOpType.bypass, replica_groups=groups,
                ins=[slice0[:]], outs=[tab1[:]])

            pool_acc = constp.tile([P, G], F32, tag="pool_acc")

            def emit_layer(L, tab, W_sb, bias_col, dequant, out_slice):
                stream_info = [
                    ("lo", NTLO, CHLO, idxlo, dofflo, tab[0:NHALF, :]),
                    ("hi", NTHI, CHHI, idxhi, doffhi, tab[NHALF:NPAD, :]),
                ]
                with tc.For_i(0, NBLK, name=f"layer{L}") as i:
                    bufs = {}
                    qn = 0
                    for sname, NT, CH, idx_sb, _, tab_ap in stream_info:
                        buf = msgp.tile([P, NT * P], BF16, tag=f"buf{sname}")
                        bufs[sname] = buf
                        a = 0       # rows done within block
                        for sz in CH:
                            nc.gpsimd.dma_gather(
                                out_ap=buf[:, a:a + sz].rearrange(
                                    "p (c f) -> p c f", f=F),
                                in_ap=tab_ap,
                                idxs_ap=idx_sb[:, ds(i * (NT * 8) + a // 16,
                                                     sz // 16)],
                                num_idxs=sz, num_idxs_reg=sz,
                                elem_size=F, single_packet=True,
                                queue_num=qn % NQ)
                            qn += 1
                            a += sz
                    agg_ps = pagg.tile([P, F], F32, space="PSUM", tag="agg")
                    ntot = NTLO + NTHI
                    wi = 0
                    for sname, NT, CH, idx_sb, doff_sb, tab_ap in stream_info:
                        buf = bufs[sname]
                        # one-hot selections built 4 tiles per DVE op:
                        # sel4[p, a, d] = (doff[p, i*NT+4g+a] == iota[d])
                        for g in range((NT + 3) // 4):
                            k0 = 4 * g
                            gsz = min(4, NT - k0)
                            sel4 = selp.tile([P, gsz * P], BF16,
                                             tag=f"sel{sname}{g}")
                            nc.vector.tensor_tensor(
                                out=sel4[:].rearrange("p (a b) -> p a b", b=P),
                                in0=doff_sb[:, ds(i * NT + k0, gsz)]
                                    .to_broadcast([P, gsz, P]),
                                in1=iota4[:, 0:gsz * P]
                                    .rearrange("p (a b) -> p a b", b=P),
                                op=mybir.AluOpType.is_equal)
                            for tt in range(gsz):
                                nc.tensor.matmul(
                                    out=agg_ps[:],
                                    lhsT=buf[:, (k0 + tt) * F:(k0 + tt + 1) * F],
                                    rhs=sel4[:, tt * P:(tt + 1) * P],
                                    start=(wi == 0),
                                    stop=(wi == ntot - 1))
                                wi += 1
                    aggT = workp.tile([P, F], BF16, tag="aggT")
                    if dequant:
                        # aggT[f, d] = agg_ps[f, d] * s_feat[f] * dinv[dst_d]
                        nc.vector.scalar_tensor_tensor(
                            out=aggT[:], in0=agg_ps[:], scalar=s_col[:, 0:1],
                            in1=dinvrep[:, ts(i, P)],
                            op0=mybir.AluOpType.mult,
                            op1=mybir.AluOpType.mult)
                    else:
                        nc.vector.tensor_copy(out=aggT[:], in_=agg_ps[:])
                    hT_ps = phT.tile([P, F], F32, space="PSUM", tag="hT")
                    nc.tensor.matmul(out=hT_ps[:], lhsT=W_sb[:], rhs=aggT[:],
                                     start=True, stop=True)
                    hT = workp.tile([P, F], F32, tag="hT_sb")
                    nc.scalar.activation(out=hT[:], in_=hT_ps[:],
                                         func=mybir.ActivationFunctionType.Relu,
                                         bias=bias_col)
                    h_ps = ptr.tile([P, F], F32, space="PSUM", tag="tr")
                    nc.tensor.transpose(out=h_ps[:], in_=hT[:], identity=ident[:])
                    h_sb = workp.tile([P, F], BF16, tag="h_sb")
                    nc.vector.tensor_copy(out=h_sb[:], in_=h_ps[:])
                    if out_slice is not None:
                        nc.sync.dma_start(out=out_slice[ts(i, P), :], in_=h_sb[:])
                    else:
                        # pool: one-hot [node -> graph] and accumulate [F, G]
                        selg = selp.tile([P, G], BF16, tag="selg")
                        nc.vector.tensor_scalar(
                            out=selg[:], in0=iotaG[:],
                            scalar1=bat[:, ds(i, 1)], scalar2=None,
                            op0=mybir.AluOpType.is_equal)
                        pmm = ppool.tile([P, G], F32, space="PSUM", tag="pmm")
                        nc.tensor.matmul(out=pmm[:], lhsT=h_sb[:], rhs=selg[:],
                                         start=True, stop=True)
                        nc.vector.tensor_add(out=pool_acc[:], in0=pool_acc[:],
                                             in1=pmm[:])

            emit_layer(0, tab1, w0, bcols[:, 0:1], True, slice1)
            nc.gpsimd.collective_compute(
                "AllGather", mybir.AluOpType.bypass, replica_groups=groups,
                ins=[slice1[:]], outs=[tab2[:]])
            emit_layer(1, tab2, wg1, bcols[:, 1:2], False, slice2)
            nc.gpsimd.collective_compute(
                "AllGather", mybir.AluOpType.bypass, replica_groups=groups,
                ins=[slice2[:]], outs=[tab3[:]])
            nc.any.memset(pool_acc[:], 0.0)
            emit_layer(2, tab3, wg2, bcols[:, 2:3], False, None)

            # ---- pooling: partial sums [F, G] -> AllReduce -> mean
            nc.sync.dma_start(out=pool_in[:], in_=pool_acc[:])
            nc.gpsimd.collective_compute(
                "AllReduce", mybir.AluOpType.add, replica_groups=groups,
                ins=[pool_in[:]], outs=[pool_out[:]])
            mT = workp.tile([P, G], F32, tag="mT")     # [F, G] mean, feat-major
            nc.sync.dma_start(out=mT[:], in_=pool_out[:])
            mTb = workp.tile([P, G], BF16, tag="mTb")
            nc.vector.tensor_mul(out=mTb[:], in0=mT[:], in1=invc_rep[:])

            # ---- head (redundant on every core), all graph-minor [*, G]
            g1T = []
            for h in range(NHID // P):
                g_ps = phead.tile([P, G], F32, space="PSUM", tag=f"ghead{h}")
                nc.tensor.matmul(out=g_ps[:], lhsT=wh1[:, h * P:(h + 1) * P],
                                 rhs=mTb[:], start=True, stop=True)
                gt = workp.tile([P, G], BF16, tag=f"g1T{h}")
                nc.scalar.activation(out=gt[:], in_=g_ps[:],
                                     func=mybir.ActivationFunctionType.Relu,
                                     bias=bcols[:, 3 + h:4 + h])
                g1T.append(gt)
            o_ps = phead.tile([P, G], F32, space="PSUM", tag="ohead")
            for h in range(NHID // P):
                nc.tensor.matmul(out=o_ps[:], lhsT=wh2[:, h * NOUT:(h + 1) * NOUT],
                                 rhs=g1T[h][:], start=(h == 0),
                                 stop=(h == NHID // P - 1))
            outT = workp.tile([P, G], F32, tag="outT")   # [NOUT, G]
            nc.vector.tensor_scalar(out=outT[:], in0=o_ps[:],
                                    scalar1=bcols[:, 5:6], scalar2=None,
                                    op0=mybir.AluOpType.add)
            # each core emits only its own GPC graphs (reassembled on host)
            pid = nc.vector.partition_id()
            oslice = workp.tile([P, GPC], F32, tag="oslice")
            nc.vector.tensor_copy(out=oslice[:], in_=outT[:, ds(pid * GPC, GPC)])
            tr_ps = ptr.tile([GPC, P], F32, space="PSUM", tag="otr")
            nc.tensor.transpose(out=tr_ps[:], in_=oslice[:], identity=ident[:])
            o_sb = workp.tile([GPC, NOUT], F32, tag="o_out")
            nc.vector.tensor_copy(out=o_sb[:], in_=tr_ps[:])
            nc.sync.dma_start(out=out_d[:], in_=o_sb[:])

    nc.compile()
    return nc


_CACHE = {}


def run(cfg, inputs):
    in_maps, meta = preprocess(cfg, **inputs)
    key = (cfg.N, meta["NTLO"], meta["NTHI"], meta["BLOB"])
    if key not in _CACHE:
        _CACHE[key] = build_program(cfg, meta)
    nc = _CACHE[key]
    res = run_bass_kernel_spmd(nc, in_maps, core_ids=list(range(NCORES)))
    return np.concatenate(
        [np.asarray(res.results[c]["out"]) for c in range(NCORES)],
        axis=0).astype(np.float32)


def kernel(**inputs):
    return run(FULL, inputs)
